# revision 32
# baseline (speedup 1.0000x reference)
"""Trainium2 Bass kernel for an Attention + dense-MoE transformer layer.

Distribution: pure data-parallel over the batch dim (B=8) across 8
NeuronCores — one batch element per core, weights replicated. The dense
MoE (every token through every expert, gate-weighted sum) means compute
is identical under any sharding; DP avoids all collectives.

Per-core pipeline (S=1024 tokens, D=1024, H=16 heads, F=4096, E=8):
  rmsnorm1 -> (PE-transpose) xnT -> QKV (q,k feature-major; v token-major)
  -> per-head scoresT = k_h^T.T@q_h^T -> exp (no max; values bounded)
  -> denom via ones-matmul (cross-partition sum, broadcast) -> av matmul
  -> scale by 1/denom -> Wo proj + residual -> rmsnorm2 -> x1nT
  -> gate softmax -> per-expert h=relu(x1n@W1e+b1), y=h@W2e, acc += g*(y+b2)
  -> out = acc (accumulated in-place on x1).

All matmul operands are bf16 (f32r self-loads its 4B weights inside the
MATMUL at ~1.5x cost; bf16 stationary operands get pipelined FWL).
Weights are DMA'd f32 and cast to bf16 on DVE/ACT/GpSimd (spread so no
engine bottlenecks). Residual stream, norms, psum accum stay f32.
"""
import sys

if '/opt/trn_rl_repo' not in sys.path:
    sys.path.insert(0, '/opt/trn_rl_repo')

import numpy as np

import concourse.bass as bass
import concourse.tile as tile
from concourse import bacc, mybir
from concourse.masks import make_identity
from concourse.bass_utils import run_bass_kernel_spmd

F32 = mybir.dt.float32
F32R = mybir.dt.float32r
BF16 = mybir.dt.bfloat16
FP8 = mybir.dt.float8e4
DR = mybir.MatmulPerfMode.DoubleRow
ALU = mybir.AluOpType
AX = mybir.AxisListType.X
AF = mybir.ActivationFunctionType

B, S, D, H, F, E = 8, 1024, 1024, 16, 4096, 8
DH = D // H            # 64 head dim
ST = S // 128          # 8 token tiles
DT = D // 128          # 8 feature tiles
FT = F // 128          # 32 ffn tiles
EPS = 1e-6
SCALE = DH ** -0.5     # 0.125
NCORES = 8
CH = 512               # attention s_q chunk
NCH = S // CH          # 4


def _bcast_row(handle, offset, n):
    """AP reading a [n] DRAM row broadcast across 128 partitions."""
    return bass.AP(tensor=handle.ap().tensor, offset=offset, ap=[[0, 128], [1, n]])


def build():
    nc = bacc.Bacc("TRN2", target_bir_lowering=False)

    x = nc.declare_dram_parameter("x", [S, D], F32, isOutput=False)
    g1 = nc.declare_dram_parameter("g1", [D], F32, isOutput=False)
    Wq = nc.declare_dram_parameter("Wq", [D, D], F32, isOutput=False)
    Wk = nc.declare_dram_parameter("Wk", [D, D], F32, isOutput=False)
    Wv = nc.declare_dram_parameter("Wv", [D, D], F32, isOutput=False)
    Wo = nc.declare_dram_parameter("Wo", [D, D], F32, isOutput=False)
    g2 = nc.declare_dram_parameter("g2", [D], F32, isOutput=False)
    Wg = nc.declare_dram_parameter("Wg", [D, E], F32, isOutput=False)
    W1 = nc.declare_dram_parameter("W1", [E, D, F], F32, isOutput=False)
    b1 = nc.declare_dram_parameter("b1", [E, F], F32, isOutput=False)
    W2 = nc.declare_dram_parameter("W2", [E, F, D], F32, isOutput=False)
    b2 = nc.declare_dram_parameter("b2", [E, D], F32, isOutput=False)
    out = nc.declare_dram_parameter("out", [S, D], F32, isOutput=True)

    def cast_dve(dst, src):
        nc.vector.tensor_copy(dst, src.bitcast(F32))

    def cast_act(dst, src):
        nc.scalar.activation(dst, src.bitcast(F32), AF.Copy)

    def cast_gps(dst, src):
        nc.gpsimd.tensor_copy(dst, src.bitcast(F32))

    with tile.TileContext(nc) as tc:
        with tc.tile_pool(name="pers", bufs=1) as pers, \
             tc.tile_pool(name="x1p", bufs=1) as x1p, \
             tc.tile_pool(name="tmp", bufs=3) as tmp, \
             tc.tile_pool(name="small", bufs=4) as small:

            # ---- persistent setup ----
            x_sb = pers.tile([128, ST, D], F32)
            xr = x.ap().rearrange("(st p) d -> p st d", p=128)
            for st in range(ST):
                nc.sync.dma_start(out=x_sb[:, st, :], in_=xr[:, st, :])
            ident = pers.tile([128, 128], F32)
            make_identity(nc, ident)
            ones_bf = pers.tile([128, 128], BF16)
            nc.vector.memset(ones_bf, 1.0)
            eps_sb = pers.tile([128, 1], F32)
            nc.vector.memset(eps_sb, EPS)
            expb_sb = pers.tile([128, 1], F32)
            nc.vector.memset(expb_sb, -5.0 * float(np.log(2.0)))
            wg_sb = pers.tile([128, DT, E], BF16)
            nc.gpsimd.dma_start(out=wg_sb, in_=Wg.ap().rearrange("(kt p) e -> p kt e", p=128))
            b1_sb = pers.tile([128, E, FT], F32)
            nc.sync.dma_start(out=b1_sb, in_=b1.ap().rearrange("e (ft p) -> p e ft", p=128))
            # h is kept as 32*relu(x@W1+b1) (still within e4m3 range) so the
            # 1/32 never touches the ACT/DVE relu path; b1 is pre-scaled to
            # match and the 1/(32*64) compensated in the gate factor.
            b1s32 = pers.tile([128, E, FT], F32)
            nc.vector.tensor_scalar_mul(b1s32, b1_sb, 32.0)
            zeros1024 = pers.tile([128, 1024], F32)
            nc.vector.memset(zeros1024, 0.0)
            gate_sb = pers.tile([128, ST, E], F32)
            x1nT = x1p.tile([128, DT, S], BF16)

            def rmsnorm_transpose(gcol, dstT, ps_tp):
                """token-major rmsnorm of x_sb then PE-transpose into dstT
                [128, DT, S] (bf16, feature-major). The norm gain g is folded
                into the transpose-drain (per-partition scale in transposed
                layout), alternating DVE/ACT."""
                for st in range(ST):
                    xs = x_sb[:, st, :]
                    sq = tmp.tile([128, D], F32, tag="scr")
                    nc.vector.tensor_mul(sq, xs, xs)
                    ss = small.tile([128, 1], F32, tag="ss")
                    nc.vector.reduce_sum(ss, sq, axis=AX)
                    rstd = small.tile([128, 1], F32, tag="rstd")
                    nc.scalar.activation(rstd, ss, AF.Sqrt, bias=eps_sb, scale=1.0 / D)
                    rs = small.tile([128, 1], F32, tag="rs")
                    nc.vector.reciprocal(rs, rstd)
                    xn = tmp.tile([128, D], F32, tag="scr")
                    nc.vector.tensor_scalar_mul(xn, xs, rs)
                    for dt_ in range(DT):
                        tp = ps_tp.tile([128, 128], F32, tag="tp", bufs=2)
                        nc.tensor.transpose(tp, xn[:, dt_ * 128:(dt_ + 1) * 128], ident)
                        dslice = dstT[:, dt_, st * 128:(st + 1) * 128]
                        if dt_ % 2 == 0:
                            nc.vector.tensor_scalar_mul(
                                dslice, tp, gcol[:, dt_:dt_ + 1])
                        else:
                            nc.scalar.activation(
                                dslice, tp, AF.Copy, scale=gcol[:, dt_:dt_ + 1])

            # ================= Scope I: attention =================
            with tc.tile_pool(name="attn", bufs=1) as attn, \
                 tc.tile_pool(name="wbigp", bufs=2) as wbigp, \
                 tc.tile_pool(name="wstg", bufs=1) as wstg, \
                 tc.tile_pool(name="gpool", bufs=1) as gpool:
                gc1 = gpool.tile([128, DT], F32)
                nc.sync.dma_start(out=gc1, in_=g1.ap().rearrange("(dt p) -> p dt", p=128))
                gc2 = gpool.tile([128, DT], F32)
                nc.sync.dma_start(out=gc2, in_=g2.ap().rearrange("(dt p) -> p dt", p=128))
                xnT = attn.tile([128, DT, S], BF16, tag="xT")
                qT = attn.tile([128, DT, S], BF16, tag="qT")
                kT = attn.tile([128, DT, S], BF16, tag="kT")
                # V in fp8 with a ones column appended per head: the AV
                # matmul then yields the softmax denominator for free in
                # psum row 64 (out rows 0:64 = sum(exp*v), row 64 = sum(exp)).
                ve_sb = attn.tile([128, ST, H, 65], FP8, tag="v")
                nc.vector.memset(ve_sb[:, :, :, 64:65], 1.0)

                with tc.tile_pool(name="ps12", bufs=2, space="PSUM") as ps12, \
                     tc.tile_pool(name="wsa", bufs=2) as wsa:
                    rmsnorm_transpose(gc1, xnT, ps12)

                    for wh, dstT in ((Wq, qT), (Wk, kT)):
                        for mt in range(DT):
                            wstage = wsa.tile([128, DT, 128], F32, tag="wstage")
                            nc.sync.dma_start(
                                out=wstage,
                                in_=wh[:, mt * 128:(mt + 1) * 128].rearrange(
                                    "(kt p) m -> p kt m", p=128))
                            wbf = wsa.tile([128, DT, 128], BF16, tag="wbf")
                            cast_dve(wbf, wstage)
                            ps = ps12.tile([128, 1024], F32, tag="mm")
                            for nh in range(2):
                                for kt in range(DT):
                                    nc.tensor.matmul(
                                        ps[:, nh * 512:(nh + 1) * 512],
                                        wbf[:, kt, :],
                                        xnT[:, kt, nh * 512:(nh + 1) * 512],
                                        start=(kt == 0), stop=(kt == DT - 1))
                            nc.vector.tensor_copy(dstT[:, mt, :], ps)

                    for nh in range(2):
                        wv_c = wbigp.tile([128, DT, 512], BF16, tag="wbig")
                        for hf in range(2):
                            wv_s = wstg.tile([128, DT, 256], F32, tag="wbigs")
                            nc.sync.dma_start(
                                out=wv_s,
                                in_=Wv[:, nh * 512 + hf * 256:nh * 512 + (hf + 1) * 256].rearrange(
                                    "(kt p) n -> p kt n", p=128))
                            cast_act(wv_c[:, :, hf * 256:(hf + 1) * 256], wv_s)
                        for st in range(ST):
                            ps = ps12.tile([128, 8, 64], F32, tag="vps")
                            for kt in range(DT):
                                nc.tensor.matmul(
                                    ps[:, 0:8, 0:64],
                                    xnT[:, kt, st * 128:(st + 1) * 128],
                                    wv_c[:, kt, :],
                                    start=(kt == 0), stop=(kt == DT - 1))
                            nc.vector.tensor_copy(
                                ve_sb[:, st, nh * 8:(nh + 1) * 8, 0:64], ps)

                # ---- attention core ----
                # Head PAIRS (2t, 2t+1) share one 128-row tile of qT/kT:
                # even head in partitions 0-63, odd in 64-127. Scores stay
                # bf16; exp is written fp8e4 (pre-scaled 2^-5 via the ACT
                # bias so the max stays under e4m3's 240) in [128,1024]
                # merged ops. AV runs fp8 DoubleRow over kt pairs with the
                # ones column yielding the denominator in psum row 64; the
                # reciprocal row is PE-broadcast across partitions.
                # Software-pipelined: iteration i's scores are emitted
                # before iteration i-1's drain.
                avT = attn.tile([128, DT, S], BF16, tag="xT")  # reuses xnT slot
                with tc.tile_pool(name="ps3", bufs=2, space="PSUM") as ps3, \
                     tc.tile_pool(name="expp", bufs=2) as expp, \
                     tc.tile_pool(name="recp", bufs=2) as recp:

                    def attn_drain(state):
                        t, cs, exp_e, exp_o = state
                        ps_e = ps3.tile([128, CH], F32, tag="ave", bufs=2)
                        ps_o = ps3.tile([128, CH], F32, tag="avo", bufs=2)
                        for kp in range(ST // 2):
                            nc.tensor.matmul(
                                ps_e[0:65, :],
                                ve_sb[:, 2 * kp:2 * kp + 2, 2 * t, :],
                                exp_e[:, 2 * kp:2 * kp + 2, :],
                                start=(kp == 0), stop=(kp == ST // 2 - 1),
                                perf_mode=DR)
                            nc.tensor.matmul(
                                ps_o[0:65, :],
                                ve_sb[:, 2 * kp:2 * kp + 2, 2 * t + 1, :],
                                exp_o[:, 2 * kp:2 * kp + 2, :],
                                start=(kp == 0), stop=(kp == ST // 2 - 1),
                                perf_mode=DR)
                        # PSUM reads are partition-window-locked on HW (a
                        # 1-lane [64:65]->[0:1] read returns wrong data), so:
                        # copy psum rows 0:65 to SBUF at natural windows,
                        # DMA-broadcast the denominator rows (DMA is
                        # partition-agnostic), take one full-width reciprocal.
                        dn_e = recp.tile([65, CH], F32, tag="dne")
                        dn_o = recp.tile([65, CH], F32, tag="dno")
                        nc.vector.tensor_copy(dn_e, ps_e[0:65, :])
                        nc.vector.tensor_copy(dn_o, ps_o[0:65, :])
                        dn_bc = recp.tile([128, CH], F32, tag="dnbc")
                        for dn, base in ((dn_e, 0), (dn_o, 64)):
                            src = dn[64:65, :]
                            nc.gpsimd.dma_start(
                                out=dn_bc[base:base + 64, :],
                                in_=bass.AP(
                                    tensor=src.tensor, offset=src.offset,
                                    ap=[src.ap[0], [0, 64], src.ap[1]]))
                        rec_bc = recp.tile([128, CH], F32, tag="rbc")
                        nc.vector.reciprocal_approx_fast(rec_bc, dn_bc)
                        # in0 from PSUM: mixed partition bases are only legal
                        # when one input is PSUM (SBUF+SBUF must align)
                        nc.vector.tensor_mul(
                            avT[0:64, t, cs], ps_e[0:64, :], rec_bc[0:64, :])
                        nc.vector.tensor_mul(
                            avT[64:128, t, cs], ps_o[0:64, :], rec_bc[64:128, :])

                    prev = None
                    for t in range(H // 2):
                        for c in range(NCH):
                            cs = slice(c * CH, (c + 1) * CH)
                            exp_e = expp.tile([128, ST, CH], FP8, tag="expe")
                            exp_o = expp.tile([128, ST, CH], FP8, tag="expo")
                            for kp in range(ST // 2):
                                for par in range(2):
                                    lo, hi = (0, 64) if par == 0 else (64, 128)
                                    dst = exp_e if par == 0 else exp_o
                                    sc = ps3.tile([128, 2, 512], F32, tag="sc")
                                    for j in range(2):
                                        ks = slice((2 * kp + j) * 128,
                                                   (2 * kp + j + 1) * 128)
                                        nc.tensor.matmul(
                                            sc[:, j, :], kT[lo:hi, t, ks],
                                            qT[lo:hi, t, cs],
                                            start=True, stop=True)
                                    nc.scalar.activation(
                                        dst[:, 2 * kp:2 * kp + 2, :], sc,
                                        AF.Exp, scale=SCALE, bias=expb_sb)
                            cur = (t, cs, exp_e, exp_o)
                            if prev is not None:
                                attn_drain(prev)
                            prev = cur
                    attn_drain(prev)

                # ---- Wo proj + residual, rmsnorm2, gate ----
                with tc.tile_pool(name="ps4", bufs=3, space="PSUM") as ps4:
                    for nh in range(2):
                        wo_c = wbigp.tile([128, DT, 512], BF16, tag="wbig")
                        for hf in range(2):
                            wo_s = wstg.tile([128, DT, 256], F32, tag="wbigs")
                            nc.sync.dma_start(
                                out=wo_s,
                                in_=Wo[:, nh * 512 + hf * 256:nh * 512 + (hf + 1) * 256].rearrange(
                                    "(kt p) n -> p kt n", p=128))
                            cast_act(wo_c[:, :, hf * 256:(hf + 1) * 256], wo_s)
                        for st in range(ST):
                            ps = ps4.tile([128, 512], F32, tag="mm")
                            for kt in range(DT):
                                nc.tensor.matmul(
                                    ps, avT[:, kt, st * 128:(st + 1) * 128],
                                    wo_c[:, kt, :],
                                    start=(kt == 0), stop=(kt == DT - 1))
                            nc.vector.tensor_add(
                                x_sb[:, st, nh * 512:(nh + 1) * 512],
                                x_sb[:, st, nh * 512:(nh + 1) * 512], ps)

                    rmsnorm_transpose(gc2, x1nT, ps4)

                    # gate = softmax(x1n @ Wg) token-major [128, st, E]
                    for st in range(ST):
                        ps = ps4.tile([128, 512], F32, tag="mm")
                        for kt in range(DT):
                            nc.tensor.matmul(
                                ps[:, :E], x1nT[:, kt, st * 128:(st + 1) * 128],
                                wg_sb[:, kt, :],
                                start=(kt == 0), stop=(kt == DT - 1))
                        gexp = small.tile([128, E], F32, tag="gexp")
                        nc.scalar.activation(gexp, ps[:, :E], AF.Exp)
                        gsum = small.tile([128, 1], F32, tag="gsum")
                        nc.vector.reduce_sum(gsum, gexp, axis=AX)
                        grec = small.tile([128, 1], F32, tag="grec")
                        nc.vector.reciprocal(grec, gsum)
                        nc.vector.tensor_scalar_mul(gate_sb[:, st, :], gexp, grec)

                    # out += gate @ b2 (handles the Sum_e g_e*b2_e term once)
                    b2rb = gpool.tile([8, D], BF16)
                    nc.gpsimd.dma_start(out=b2rb, in_=b2.ap())
                    gateT = gpool.tile([8, ST, 128], BF16)
                    for st in range(ST):
                        tpg = ps4.tile([128, 128], F32, tag="tp", bufs=2)
                        nc.tensor.transpose(
                            tpg[:8, :], gate_sb[:, st, :], ident)
                        nc.vector.tensor_copy(gateT[:, st, :], tpg[:8, :])
                    for st in range(ST):
                        for nh in range(2):
                            ps = ps4.tile([128, 512], F32, tag="mm")
                            nc.tensor.matmul(
                                ps, gateT[:, st, :],
                                b2rb[:, nh * 512:(nh + 1) * 512],
                                start=True, stop=True)
                            nc.vector.tensor_add(
                                x_sb[:, st, nh * 512:(nh + 1) * 512],
                                x_sb[:, st, nh * 512:(nh + 1) * 512], ps)

            # ================= Scope II: MoE (fp8e4 DoubleRow) =================
            # Both operands of both expert matmuls are e4m3, contracted two
            # k-rows per PE pass (perf_mode=DoubleRow). Weights pre-scaled
            # into e4m3's range (32*W1 std->1, 64*W2 std->1), compensated
            # downstream: h = relu(ps/32 + b1) via ACT scale; y-accum scales
            # by gate/64. W1/W2 each DMA'd once per expert (f32), quantized
            # on DVE/GpSimd while the previous expert's matmuls run.
            with tc.tile_pool(name="moe", bufs=1) as moe, \
                 tc.tile_pool(name="hp", bufs=1) as hp, \
                 tc.tile_pool(name="w1p", bufs=2) as w1p, \
                 tc.tile_pool(name="w2p", bufs=2) as w2p, \
                 tc.tile_pool(name="ps5", bufs=4, space="PSUM") as ps5:

                x1n8 = moe.tile([128, DT, S], FP8, tag="x1n8")
                for dt_ in range(DT):
                    if dt_ % 2 == 1:
                        nc.scalar.activation(
                            x1n8[:, dt_, :], x1nT[:, dt_, :], AF.Copy)
                    else:
                        nc.vector.tensor_copy(x1n8[:, dt_, :], x1nT[:, dt_, :])
                gate64 = moe.tile([128, ST, E], F32, tag="g64")
                nc.vector.tensor_scalar_mul(gate64, gate_sb, 1.0 / 2048.0)

                w1q = moe.tile([128, DT, F], FP8, tag="w1q")
                w2q = moe.tile([128, FT, D], FP8, tag="w2q")
                hT = hp.tile([128, FT, S], FP8, tag="hT")

                for e in range(E):
                    # ---- stage + quantize W1[e]: [D, F] -> [p, kt, f]*32 ----
                    for fc in range(16):
                        w1s = w1p.tile([128, DT, 256], F32, tag="w1s")
                        nc.sync.dma_start(
                            out=w1s,
                            in_=W1[e, :, fc * 256:(fc + 1) * 256].rearrange(
                                "(kt p) f -> p kt f", p=128))
                        dst = w1q[:, :, fc * 256:(fc + 1) * 256]
                        nc.vector.tensor_scalar_mul(dst, w1s, 32.0)
                    # ---- stage + quantize W2[e]: [F, D] -> [p, ft, d]*64 ----
                    for wc in range(FT):
                        w2s = w2p.tile([128, D], F32, tag="w2s")
                        nc.sync.dma_start(
                            out=w2s, in_=W2[e, wc * 128:(wc + 1) * 128, :])
                        if wc % 4 != 3:
                            nc.vector.tensor_scalar_mul(w2q[:, wc, :], w2s, 64.0)
                        else:
                            nc.scalar.activation(
                                w2q[:, wc, :], w2s, AF.Copy, scale=64.0)
                    # ---- h' = 32*relu(x1n @ W1e + b1), all S tokens ----
                    # relu split across ACT and DVE so neither gates the
                    # h-phase (ACT alone was 3x the PE time here).
                    for fb in range(FT):
                        ps_h = ps5.tile([128, 1024], F32, tag="h", bufs=2)
                        for sh in range(2):
                            for kp in range(DT // 2):
                                nc.tensor.matmul(
                                    ps_h[:, sh * 512:(sh + 1) * 512],
                                    w1q[:, 2 * kp:2 * kp + 2,
                                        fb * 128:(fb + 1) * 128],
                                    x1n8[:, 2 * kp:2 * kp + 2,
                                         sh * 512:(sh + 1) * 512],
                                    start=(kp == 0), stop=(kp == DT // 2 - 1),
                                    perf_mode=DR)
                        hdst = hT[:, fb, :]
                        if fb % 2 == 0:
                            nc.scalar.activation(
                                hdst, ps_h, AF.Relu,
                                bias=b1s32[:, e, fb:fb + 1], scale=1.0)
                        else:
                            nc.vector.scalar_tensor_tensor(
                                hdst, ps_h, b1s32[:, e, fb:fb + 1],
                                zeros1024, ALU.add, ALU.max)
                    # ---- y = h @ W2e; x_sb += (gate/64) * y ----
                    for st in range(ST):
                        for nh in range(2):
                            ps_y = ps5.tile([128, 512], F32, tag="y")
                            for fp_ in range(FT // 2):
                                nc.tensor.matmul(
                                    ps_y,
                                    hT[:, 2 * fp_:2 * fp_ + 2,
                                       st * 128:(st + 1) * 128],
                                    w2q[:, 2 * fp_:2 * fp_ + 2,
                                        nh * 512:(nh + 1) * 512],
                                    start=(fp_ == 0), stop=(fp_ == FT // 2 - 1),
                                    perf_mode=DR)
                            xs = x_sb[:, st, nh * 512:(nh + 1) * 512]
                            nc.vector.scalar_tensor_tensor(
                                xs, ps_y, gate64[:, st, e:e + 1], xs,
                                ALU.mult, ALU.add)

            outr = out.ap().rearrange("(st p) d -> p st d", p=128)
            for st in range(ST):
                nc.sync.dma_start(out=outr[:, st, :], in_=x_sb[:, st, :])

    nc.finalize()
    return nc


_CACHE = {}


def _get_nc():
    if 'nc' not in _CACHE:
        _CACHE['nc'] = build()
    return _CACHE['nc']


def _in_maps(inputs):
    xf = np.ascontiguousarray(np.asarray(inputs['x'], dtype=np.float32))
    assert xf.shape == (B, S, D)
    nh = inputs.get('n_heads', H)
    assert int(nh) == H, f"kernel hardcodes n_heads={H}, got {nh}"
    base = {}
    for k in ('g1', 'Wq', 'Wk', 'Wv', 'Wo', 'g2', 'Wg', 'W1', 'b1', 'W2', 'b2'):
        base[k] = np.ascontiguousarray(np.asarray(inputs[k], dtype=np.float32))
    return [dict(base, x=xf[i]) for i in range(NCORES)]


def kernel(**inputs):
    nc = _get_nc()
    res = run_bass_kernel_spmd(nc, _in_maps(inputs), core_ids=list(range(NCORES)))
    return np.stack([res.results[i]['out'] for i in range(NCORES)], axis=0)


def kernel_profiled(**inputs):
    """Like kernel() but also returns neuron-profile exec_time_ns."""
    import os
    import pickle
    import shutil
    tdir = '/tmp/trn_trace'
    shutil.rmtree(tdir, ignore_errors=True)
    os.makedirs(tdir, exist_ok=True)
    nc = _get_nc()
    res = run_bass_kernel_spmd(
        nc, _in_maps(inputs), core_ids=list(range(NCORES)),
        trace=True, tmpdir=tdir)
    outv = np.stack([res.results[i]['out'] for i in range(NCORES)], axis=0)
    if res.instructions_and_trace is not None:
        insts, tpath = res.instructions_and_trace
        try:
            with open(os.path.join(tdir, 'insts.pkl'), 'wb') as f:
                pickle.dump(insts, f)
        except Exception as e:
            print(f"[kernel_profiled] inst pickle failed: {e}")
        print(f"[kernel_profiled] trace: {tpath}")
    return outv, res.exec_time_ns



# revision 39
# speedup vs baseline: 1.2369x; 1.2369x over previous
"""Trainium2 Bass kernel for an Attention + dense-MoE transformer layer.

Distribution: pure data-parallel over the batch dim (B=8) across 8
NeuronCores — one batch element per core, weights replicated. The dense
MoE (every token through every expert, gate-weighted sum) means compute
is identical under any sharding; DP avoids all collectives.

Per-core pipeline (S=1024 tokens, D=1024, H=16 heads, F=4096, E=8):
  rmsnorm1 -> (PE-transpose) xnT -> QKV (q,k feature-major; v token-major)
  -> per-head scoresT = k_h^T.T@q_h^T -> exp (no max; values bounded)
  -> denom via ones-matmul (cross-partition sum, broadcast) -> av matmul
  -> scale by 1/denom -> Wo proj + residual -> rmsnorm2 -> x1nT
  -> gate softmax -> per-expert h=relu(x1n@W1e+b1), y=h@W2e, acc += g*(y+b2)
  -> out = acc (accumulated in-place on x1).

All matmul operands are bf16 (f32r self-loads its 4B weights inside the
MATMUL at ~1.5x cost; bf16 stationary operands get pipelined FWL).
Weights are DMA'd f32 and cast to bf16 on DVE/ACT/GpSimd (spread so no
engine bottlenecks). Residual stream, norms, psum accum stay f32.
"""
import sys

if '/opt/trn_rl_repo' not in sys.path:
    sys.path.insert(0, '/opt/trn_rl_repo')

import numpy as np

import concourse.bass as bass
import concourse.tile as tile
from concourse import bacc, mybir
from concourse.masks import make_identity
from concourse.bass_utils import run_bass_kernel_spmd

F32 = mybir.dt.float32
F32R = mybir.dt.float32r
BF16 = mybir.dt.bfloat16
FP8 = mybir.dt.float8e4
DR = mybir.MatmulPerfMode.DoubleRow
ALU = mybir.AluOpType
AX = mybir.AxisListType.X
AF = mybir.ActivationFunctionType

B, S, D, H, F, E = 8, 1024, 1024, 16, 4096, 8
DH = D // H            # 64 head dim
ST = S // 128          # 8 token tiles
DT = D // 128          # 8 feature tiles
FT = F // 128          # 32 ffn tiles
EPS = 1e-6
SCALE = DH ** -0.5     # 0.125
NCORES = 8
CH = 512               # attention s_q chunk
NCH = S // CH          # 4


def _bcast_row(handle, offset, n):
    """AP reading a [n] DRAM row broadcast across 128 partitions."""
    return bass.AP(tensor=handle.ap().tensor, offset=offset, ap=[[0, 128], [1, n]])


def build():
    nc = bacc.Bacc("TRN2", target_bir_lowering=False)

    x = nc.declare_dram_parameter("x", [S, D], F32, isOutput=False)
    g1 = nc.declare_dram_parameter("g1", [D], F32, isOutput=False)
    Wq = nc.declare_dram_parameter("Wq", [D, D], F32, isOutput=False)
    Wk = nc.declare_dram_parameter("Wk", [D, D], F32, isOutput=False)
    Wv = nc.declare_dram_parameter("Wv", [D, D], F32, isOutput=False)
    Wo = nc.declare_dram_parameter("Wo", [D, D], F32, isOutput=False)
    g2 = nc.declare_dram_parameter("g2", [D], F32, isOutput=False)
    Wg = nc.declare_dram_parameter("Wg", [D, E], F32, isOutput=False)
    W1 = nc.declare_dram_parameter("W1", [E, D, F], F32, isOutput=False)
    b1 = nc.declare_dram_parameter("b1", [E, F], F32, isOutput=False)
    W2 = nc.declare_dram_parameter("W2", [E, F, D], F32, isOutput=False)
    b2 = nc.declare_dram_parameter("b2", [E, D], F32, isOutput=False)
    out = nc.declare_dram_parameter("out", [S, D], F32, isOutput=True)

    def cast_dve(dst, src):
        nc.vector.tensor_copy(dst, src.bitcast(F32))

    def cast_act(dst, src):
        nc.scalar.activation(dst, src.bitcast(F32), AF.Copy)

    def cast_gps(dst, src):
        nc.gpsimd.tensor_copy(dst, src.bitcast(F32))

    with tile.TileContext(nc) as tc:
        with tc.tile_pool(name="pers", bufs=1) as pers, \
             tc.tile_pool(name="x1p", bufs=1) as x1p, \
             tc.tile_pool(name="tmp", bufs=3) as tmp, \
             tc.tile_pool(name="small", bufs=4) as small:

            # ---- persistent setup ----
            x_sb = pers.tile([128, ST, D], F32)
            xr = x.ap().rearrange("(st p) d -> p st d", p=128)
            for st in range(ST):
                nc.sync.dma_start(out=x_sb[:, st, :], in_=xr[:, st, :])
            ident = pers.tile([128, 128], F32)
            make_identity(nc, ident)
            ones_bf = pers.tile([128, 128], BF16)
            nc.vector.memset(ones_bf, 1.0)
            eps_sb = pers.tile([128, 1], F32)
            nc.vector.memset(eps_sb, EPS)
            expb_sb = pers.tile([128, 1], F32)
            nc.vector.memset(expb_sb, -5.0 * float(np.log(2.0)))
            wg_sb = pers.tile([128, DT, E], BF16)
            nc.gpsimd.dma_start(out=wg_sb, in_=Wg.ap().rearrange("(kt p) e -> p kt e", p=128))
            b1_sb = pers.tile([128, E, FT], F32)
            nc.sync.dma_start(out=b1_sb, in_=b1.ap().rearrange("e (ft p) -> p e ft", p=128))
            # h is kept as 32*relu(x@W1+b1) (still within e4m3 range) so the
            # 1/32 never touches the ACT/DVE relu path; b1 is pre-scaled to
            # match and the 1/(32*64) compensated in the gate factor.
            b1s32 = pers.tile([128, E, FT], F32)
            nc.vector.tensor_scalar_mul(b1s32, b1_sb, 32.0)
            zeros1024 = pers.tile([128, 1024], F32)
            nc.vector.memset(zeros1024, 0.0)
            gate_sb = pers.tile([128, ST, E], F32)
            x1nT = x1p.tile([128, DT, S], BF16)

            def rmsnorm_transpose(gcol, dstT, ps_tp):
                """token-major rmsnorm of x_sb then PE-transpose into dstT
                [128, DT, S] (bf16, feature-major). The norm gain g is folded
                into the transpose-drain (per-partition scale in transposed
                layout), alternating DVE/ACT."""
                for st in range(ST):
                    xs = x_sb[:, st, :]
                    sq = tmp.tile([128, D], F32, tag="scr")
                    nc.vector.tensor_mul(sq, xs, xs)
                    ss = small.tile([128, 1], F32, tag="ss")
                    nc.vector.reduce_sum(ss, sq, axis=AX)
                    rstd = small.tile([128, 1], F32, tag="rstd")
                    nc.scalar.activation(rstd, ss, AF.Sqrt, bias=eps_sb, scale=1.0 / D)
                    rs = small.tile([128, 1], F32, tag="rs")
                    nc.vector.reciprocal(rs, rstd)
                    xn = tmp.tile([128, D], F32, tag="scr")
                    nc.vector.tensor_scalar_mul(xn, xs, rs)
                    for dt_ in range(DT):
                        tp = ps_tp.tile([128, 128], F32, tag="tp", bufs=2)
                        nc.tensor.transpose(tp, xn[:, dt_ * 128:(dt_ + 1) * 128], ident)
                        dslice = dstT[:, dt_, st * 128:(st + 1) * 128]
                        if dt_ % 2 == 0:
                            nc.vector.tensor_scalar_mul(
                                dslice, tp, gcol[:, dt_:dt_ + 1])
                        else:
                            nc.scalar.activation(
                                dslice, tp, AF.Copy, scale=gcol[:, dt_:dt_ + 1])

            # ================= Scope I: attention =================
            with tc.tile_pool(name="attn", bufs=1) as attn, \
                 tc.tile_pool(name="wbigp", bufs=2) as wbigp, \
                 tc.tile_pool(name="wstg", bufs=1) as wstg, \
                 tc.tile_pool(name="gpool", bufs=1) as gpool:
                gc1 = gpool.tile([128, DT], F32)
                nc.gpsimd.dma_start(out=gc1, in_=g1.ap().rearrange("(dt p) -> p dt", p=128))
                gc2 = gpool.tile([128, DT], F32)
                nc.gpsimd.dma_start(out=gc2, in_=g2.ap().rearrange("(dt p) -> p dt", p=128))
                xnT = attn.tile([128, DT, S], BF16, tag="xT")
                qT = attn.tile([128, DT, S], BF16, tag="qT")
                kT = attn.tile([128, DT, S], BF16, tag="kT")
                # V in fp8 with a ones column appended per head: the AV
                # matmul then yields the softmax denominator for free in
                # psum row 64 (out rows 0:64 = sum(exp*v), row 64 = sum(exp)).
                ve_sb = attn.tile([128, ST, H, 65], FP8, tag="v")
                nc.vector.memset(ve_sb[:, :, :, 64:65], 1.0)

                with tc.tile_pool(name="ps12", bufs=2, space="PSUM") as ps12, \
                     tc.tile_pool(name="wsa", bufs=2) as wsa:
                    rmsnorm_transpose(gc1, xnT, ps12)

                    for wh, dstT in ((Wq, qT), (Wk, kT)):
                        for mt in range(DT):
                            wstage = wsa.tile([128, DT, 128], F32, tag="wstage")
                            nc.sync.dma_start(
                                out=wstage,
                                in_=wh[:, mt * 128:(mt + 1) * 128].rearrange(
                                    "(kt p) m -> p kt m", p=128))
                            wbf = wsa.tile([128, DT, 128], BF16, tag="wbf")
                            cast_dve(wbf, wstage)
                            ps = ps12.tile([128, 1024], F32, tag="mm")
                            for nh in range(2):
                                for kt in range(DT):
                                    nc.tensor.matmul(
                                        ps[:, nh * 512:(nh + 1) * 512],
                                        wbf[:, kt, :],
                                        xnT[:, kt, nh * 512:(nh + 1) * 512],
                                        start=(kt == 0), stop=(kt == DT - 1))
                            nc.vector.tensor_copy(dstT[:, mt, :], ps)

                    for nh in range(2):
                        wv_c = wbigp.tile([128, DT, 512], BF16, tag="wbig")
                        for hf in range(2):
                            wv_s = wstg.tile([128, DT, 256], F32, tag="wbigs")
                            nc.sync.dma_start(
                                out=wv_s,
                                in_=Wv[:, nh * 512 + hf * 256:nh * 512 + (hf + 1) * 256].rearrange(
                                    "(kt p) n -> p kt n", p=128))
                            cast_act(wv_c[:, :, hf * 256:(hf + 1) * 256], wv_s)
                        for st in range(ST):
                            ps = ps12.tile([128, 8, 64], F32, tag="vps")
                            for kt in range(DT):
                                nc.tensor.matmul(
                                    ps[:, 0:8, 0:64],
                                    xnT[:, kt, st * 128:(st + 1) * 128],
                                    wv_c[:, kt, :],
                                    start=(kt == 0), stop=(kt == DT - 1))
                            nc.vector.tensor_copy(
                                ve_sb[:, st, nh * 8:(nh + 1) * 8, 0:64], ps)

                # ---- attention core ----
                # Head PAIRS (2t, 2t+1) share one 128-row tile of qT/kT:
                # even head in partitions 0-63, odd in 64-127. Scores stay
                # bf16; exp is written fp8e4 (pre-scaled 2^-5 via the ACT
                # bias so the max stays under e4m3's 240) in [128,1024]
                # merged ops. AV runs fp8 DoubleRow over kt pairs with the
                # ones column yielding the denominator in psum row 64; the
                # reciprocal row is PE-broadcast across partitions.
                # Software-pipelined: iteration i's scores are emitted
                # before iteration i-1's drain.
                avT = attn.tile([128, DT, S], BF16, tag="xT")  # reuses xnT slot
                with tc.tile_pool(name="ps3", bufs=2, space="PSUM") as ps3, \
                     tc.tile_pool(name="expp", bufs=2) as expp, \
                     tc.tile_pool(name="recp", bufs=2) as recp:

                    def attn_drain(state):
                        t, cs, exp_e, exp_o = state
                        ps_e = ps3.tile([128, CH], F32, tag="ave", bufs=2)
                        ps_o = ps3.tile([128, CH], F32, tag="avo", bufs=2)
                        for kp in range(ST // 2):
                            nc.tensor.matmul(
                                ps_e[0:65, :],
                                ve_sb[:, 2 * kp:2 * kp + 2, 2 * t, :],
                                exp_e[:, 2 * kp:2 * kp + 2, :],
                                start=(kp == 0), stop=(kp == ST // 2 - 1),
                                perf_mode=DR)
                            nc.tensor.matmul(
                                ps_o[0:65, :],
                                ve_sb[:, 2 * kp:2 * kp + 2, 2 * t + 1, :],
                                exp_o[:, 2 * kp:2 * kp + 2, :],
                                start=(kp == 0), stop=(kp == ST // 2 - 1),
                                perf_mode=DR)
                        # PSUM reads are partition-window-locked on HW (a
                        # 1-lane [64:65]->[0:1] read returns wrong data), so:
                        # copy psum rows 0:65 to SBUF at natural windows,
                        # DMA-broadcast the denominator rows (DMA is
                        # partition-agnostic), take one full-width reciprocal.
                        dn_e = recp.tile([65, CH], F32, tag="dne")
                        dn_o = recp.tile([65, CH], F32, tag="dno")
                        nc.vector.tensor_copy(dn_e, ps_e[0:65, :])
                        nc.vector.tensor_copy(dn_o, ps_o[0:65, :])
                        dn_bc = recp.tile([128, CH], F32, tag="dnbc")
                        for dn, base in ((dn_e, 0), (dn_o, 64)):
                            src = dn[64:65, :]
                            nc.gpsimd.dma_start(
                                out=dn_bc[base:base + 64, :],
                                in_=bass.AP(
                                    tensor=src.tensor, offset=src.offset,
                                    ap=[src.ap[0], [0, 64], src.ap[1]]))
                        rec_bc = recp.tile([128, CH], F32, tag="rbc")
                        nc.vector.reciprocal_approx_fast(rec_bc, dn_bc)
                        # in0 from PSUM: mixed partition bases are only legal
                        # when one input is PSUM (SBUF+SBUF must align)
                        nc.vector.tensor_mul(
                            avT[0:64, t, cs], ps_e[0:64, :], rec_bc[0:64, :])
                        nc.vector.tensor_mul(
                            avT[64:128, t, cs], ps_o[0:64, :], rec_bc[64:128, :])

                    prev = None
                    for t in range(H // 2):
                        for c in range(NCH):
                            cs = slice(c * CH, (c + 1) * CH)
                            exp_e = expp.tile([128, ST, CH], FP8, tag="expe")
                            exp_o = expp.tile([128, ST, CH], FP8, tag="expo")
                            for kp in range(ST // 2):
                                for par in range(2):
                                    lo, hi = (0, 64) if par == 0 else (64, 128)
                                    dst = exp_e if par == 0 else exp_o
                                    sc = ps3.tile([128, 2, 512], F32, tag="sc")
                                    for j in range(2):
                                        ks = slice((2 * kp + j) * 128,
                                                   (2 * kp + j + 1) * 128)
                                        nc.tensor.matmul(
                                            sc[:, j, :], kT[lo:hi, t, ks],
                                            qT[lo:hi, t, cs],
                                            start=True, stop=True)
                                    nc.scalar.activation(
                                        dst[:, 2 * kp:2 * kp + 2, :], sc,
                                        AF.Exp, scale=SCALE, bias=expb_sb)
                            cur = (t, cs, exp_e, exp_o)
                            if prev is not None:
                                attn_drain(prev)
                            prev = cur
                    attn_drain(prev)

                # ---- Wo proj + residual, rmsnorm2, gate ----
                with tc.tile_pool(name="ps4", bufs=3, space="PSUM") as ps4:
                    for nh in range(2):
                        wo_c = wbigp.tile([128, DT, 512], BF16, tag="wbig")
                        for hf in range(2):
                            wo_s = wstg.tile([128, DT, 256], F32, tag="wbigs")
                            nc.sync.dma_start(
                                out=wo_s,
                                in_=Wo[:, nh * 512 + hf * 256:nh * 512 + (hf + 1) * 256].rearrange(
                                    "(kt p) n -> p kt n", p=128))
                            cast_act(wo_c[:, :, hf * 256:(hf + 1) * 256], wo_s)
                        for st in range(ST):
                            ps = ps4.tile([128, 512], F32, tag="mm")
                            for kt in range(DT):
                                nc.tensor.matmul(
                                    ps, avT[:, kt, st * 128:(st + 1) * 128],
                                    wo_c[:, kt, :],
                                    start=(kt == 0), stop=(kt == DT - 1))
                            nc.vector.tensor_add(
                                x_sb[:, st, nh * 512:(nh + 1) * 512],
                                x_sb[:, st, nh * 512:(nh + 1) * 512], ps)

                    rmsnorm_transpose(gc2, x1nT, ps4)

                    # gate = softmax(x1n @ Wg) token-major [128, st, E]
                    for st in range(ST):
                        ps = ps4.tile([128, 512], F32, tag="mm")
                        for kt in range(DT):
                            nc.tensor.matmul(
                                ps[:, :E], x1nT[:, kt, st * 128:(st + 1) * 128],
                                wg_sb[:, kt, :],
                                start=(kt == 0), stop=(kt == DT - 1))
                        gexp = small.tile([128, E], F32, tag="gexp")
                        nc.scalar.activation(gexp, ps[:, :E], AF.Exp)
                        gsum = small.tile([128, 1], F32, tag="gsum")
                        nc.vector.reduce_sum(gsum, gexp, axis=AX)
                        grec = small.tile([128, 1], F32, tag="grec")
                        nc.vector.reciprocal(grec, gsum)
                        nc.vector.tensor_scalar_mul(gate_sb[:, st, :], gexp, grec)

                    # out += gate @ b2 (handles the Sum_e g_e*b2_e term once)
                    b2rb = gpool.tile([8, D], BF16)
                    nc.gpsimd.dma_start(out=b2rb, in_=b2.ap())
                    gateT = gpool.tile([8, ST, 128], BF16)
                    for st in range(ST):
                        tpg = ps4.tile([128, 128], F32, tag="tp", bufs=2)
                        nc.tensor.transpose(
                            tpg[:8, :], gate_sb[:, st, :], ident)
                        nc.vector.tensor_copy(gateT[:, st, :], tpg[:8, :])
                    for st in range(ST):
                        for nh in range(2):
                            ps = ps4.tile([128, 512], F32, tag="mm")
                            nc.tensor.matmul(
                                ps, gateT[:, st, :],
                                b2rb[:, nh * 512:(nh + 1) * 512],
                                start=True, stop=True)
                            nc.vector.tensor_add(
                                x_sb[:, st, nh * 512:(nh + 1) * 512],
                                x_sb[:, st, nh * 512:(nh + 1) * 512], ps)

            # ================= Scope II: MoE (fp8e4 DoubleRow) =================
            # Both operands of both expert matmuls are e4m3, contracted two
            # k-rows per PE pass (perf_mode=DoubleRow). Weights pre-scaled
            # into e4m3's range (32*W1 std->1, 64*W2 std->1), compensated
            # downstream: h = relu(ps/32 + b1) via ACT scale; y-accum scales
            # by gate/64. W1/W2 each DMA'd once per expert (f32), quantized
            # on DVE/GpSimd while the previous expert's matmuls run.
            with tc.tile_pool(name="moe", bufs=1) as moe, \
                 tc.tile_pool(name="hp", bufs=1) as hp, \
                 tc.tile_pool(name="w1p", bufs=2) as w1p, \
                 tc.tile_pool(name="w2p", bufs=3) as w2p, \
                 tc.tile_pool(name="ps5", bufs=4, space="PSUM") as ps5:

                x1n8 = moe.tile([128, DT, S], FP8, tag="x1n8")
                for dt_ in range(DT):
                    if dt_ % 2 == 1:
                        nc.scalar.activation(
                            x1n8[:, dt_, :], x1nT[:, dt_, :], AF.Copy)
                    else:
                        nc.vector.tensor_copy(x1n8[:, dt_, :], x1nT[:, dt_, :])
                gate64 = moe.tile([128, ST, E], F32, tag="g64")
                nc.vector.tensor_scalar_mul(gate64, gate_sb, 1.0 / 2048.0)

                w1q = moe.tile([128, DT, F], FP8, tag="w1q")
                w2q = moe.tile([128, FT, D], FP8, tag="w2q")
                hT = hp.tile([128, FT, S], FP8, tag="hT")

                for e in range(E):
                    # ---- stage + quantize W1[e]: [D, F] -> [p, kt, f]*32 ----
                    # W1 staging rides the (otherwise idle) GpSimd DMA queue
                    # so it is not stuck behind W2 DMAs that block on the
                    # previous expert's y-phase freeing w2q.
                    for fc in range(16):
                        w1s = w1p.tile([128, DT, 256], F32, tag="w1s")
                        nc.gpsimd.dma_start(
                            out=w1s,
                            in_=W1[e, :, fc * 256:(fc + 1) * 256].rearrange(
                                "(kt p) f -> p kt f", p=128))
                        dst = w1q[:, :, fc * 256:(fc + 1) * 256]
                        nc.vector.tensor_scalar_mul(dst, w1s, 32.0)
                    # ---- stage + quantize W2[e]: [F, D] -> [p, ft, d]*64 ----
                    for wc in range(FT):
                        w2s = w2p.tile([128, D], F32, tag="w2s")
                        nc.sync.dma_start(
                            out=w2s, in_=W2[e, wc * 128:(wc + 1) * 128, :])
                        if wc % 4 != 3:
                            nc.vector.tensor_scalar_mul(w2q[:, wc, :], w2s, 64.0)
                        else:
                            nc.scalar.activation(
                                w2q[:, wc, :], w2s, AF.Copy, scale=64.0)
                    # ---- h' = 32*relu(x1n @ W1e + b1), all S tokens ----
                    # relu split across ACT and DVE so neither gates the
                    # h-phase (ACT alone was 3x the PE time here).
                    for fb in range(FT):
                        for sh in range(2):
                            ps_h = ps5.tile([128, 512], F32, tag="h")
                            for kp in range(DT // 2):
                                nc.tensor.matmul(
                                    ps_h,
                                    w1q[:, 2 * kp:2 * kp + 2,
                                        fb * 128:(fb + 1) * 128],
                                    x1n8[:, 2 * kp:2 * kp + 2,
                                         sh * 512:(sh + 1) * 512],
                                    start=(kp == 0), stop=(kp == DT // 2 - 1),
                                    perf_mode=DR)
                            hdst = hT[:, fb, sh * 512:(sh + 1) * 512]
                            if fb % 2 == 0:
                                nc.scalar.activation(
                                    hdst, ps_h, AF.Relu,
                                    bias=b1s32[:, e, fb:fb + 1], scale=1.0)
                            else:
                                nc.vector.scalar_tensor_tensor(
                                    hdst, ps_h, b1s32[:, e, fb:fb + 1],
                                    zeros1024[:, 0:512], ALU.add, ALU.max)
                    # ---- y = h @ W2e; x_sb += (gate/64) * y ----
                    for st in range(ST):
                        for nh in range(2):
                            ps_y = ps5.tile([128, 512], F32, tag="y")
                            for fp_ in range(FT // 2):
                                nc.tensor.matmul(
                                    ps_y,
                                    hT[:, 2 * fp_:2 * fp_ + 2,
                                       st * 128:(st + 1) * 128],
                                    w2q[:, 2 * fp_:2 * fp_ + 2,
                                        nh * 512:(nh + 1) * 512],
                                    start=(fp_ == 0), stop=(fp_ == FT // 2 - 1),
                                    perf_mode=DR)
                            xs = x_sb[:, st, nh * 512:(nh + 1) * 512]
                            nc.vector.scalar_tensor_tensor(
                                xs, ps_y, gate64[:, st, e:e + 1], xs,
                                ALU.mult, ALU.add)

            outr = out.ap().rearrange("(st p) d -> p st d", p=128)
            for st in range(ST):
                nc.sync.dma_start(out=outr[:, st, :], in_=x_sb[:, st, :])

    nc.finalize()
    return nc


_CACHE = {}


def _get_nc():
    if 'nc' not in _CACHE:
        _CACHE['nc'] = build()
    return _CACHE['nc']


def _in_maps(inputs):
    xf = np.ascontiguousarray(np.asarray(inputs['x'], dtype=np.float32))
    assert xf.shape == (B, S, D)
    nh = inputs.get('n_heads', H)
    assert int(nh) == H, f"kernel hardcodes n_heads={H}, got {nh}"
    base = {}
    for k in ('g1', 'Wq', 'Wk', 'Wv', 'Wo', 'g2', 'Wg', 'W1', 'b1', 'W2', 'b2'):
        base[k] = np.ascontiguousarray(np.asarray(inputs[k], dtype=np.float32))
    return [dict(base, x=xf[i]) for i in range(NCORES)]


def kernel(**inputs):
    nc = _get_nc()
    res = run_bass_kernel_spmd(nc, _in_maps(inputs), core_ids=list(range(NCORES)))
    return np.stack([res.results[i]['out'] for i in range(NCORES)], axis=0)


def kernel_profiled(**inputs):
    """Like kernel() but also returns neuron-profile exec_time_ns."""
    import os
    import pickle
    import shutil
    tdir = '/tmp/trn_trace'
    shutil.rmtree(tdir, ignore_errors=True)
    os.makedirs(tdir, exist_ok=True)
    nc = _get_nc()
    res = run_bass_kernel_spmd(
        nc, _in_maps(inputs), core_ids=list(range(NCORES)),
        trace=True, tmpdir=tdir)
    outv = np.stack([res.results[i]['out'] for i in range(NCORES)], axis=0)
    if res.instructions_and_trace is not None:
        insts, tpath = res.instructions_and_trace
        try:
            with open(os.path.join(tdir, 'insts.pkl'), 'wb') as f:
                pickle.dump(insts, f)
        except Exception as e:
            print(f"[kernel_profiled] inst pickle failed: {e}")
        print(f"[kernel_profiled] trace: {tpath}")
    return outv, res.exec_time_ns

